# revision 34
# baseline (speedup 1.0000x reference)
"""Trainium2 Bass kernel for nn_DGT_6485400616966 (soft decision tree forward).

Math (forward pass only):
  pred_z = x @ W_pred.T + b_pred                      [B, 1023]
  The straight-through/one-hot structure collapses: the output depends only on
  the argmax leaf of the tree AND layer, which equals a 10-level tree descent
  following sign(pred_z) at visited nodes (left if z >= 0).
  out = softmax(W_or[:, leaf]) ; std = clip(action_stds[:, leaf], -20, 2)

Device algorithm per core (8192 samples, data-parallel over 8 cores):
  1. PE: z = x @ W_pred.T via fp32r matmuls (x tile stationary, W^T moving),
     nodes padded to 1024.
  2. ACT: sigma = Sign(z) evicted PSUM->SBUF as bf16 in node-major/btile-minor
     layout S[128, 1024, 16] per 16-btile chunk.
  3. DVE/GPSIMD: u = -0.5*sigma + 0.5 in {0,1}   (u=1 -> go right)
  4. DVE: bottom-up tree collapse, r_i = r_even + u_i*(K + r_odd - r_even),
     fp16, all operands packed (btile-minor innermost) for the 2x mode.
  5. GPSIMD ap_gather: table lookup T[class, leaf] with classes replicated on
     partitions (16-partition groups share indices = same sample's leaf).
  6. PE transpose + contiguous DMA to [8192, 16] outputs.
"""

import sys

for _p in ("/opt/trn_rl_repo",):
    if _p not in sys.path:
        sys.path.insert(0, _p)

from contextlib import ExitStack

import numpy as np

import concourse.bacc as bacc
import concourse.bass as bass
import concourse.tile as tile
from concourse import mybir
from concourse.bass_utils import run_bass_kernel_spmd

HEIGHT = 10
IN_DIM = 256
OUT_DIM = 16
BATCH = 65536
N_CORES = 8
B_LOC = BATCH // N_CORES          # 8192 samples per core
NT = B_LOC // 128                 # 64 batch tiles of 128 samples
NB = 8                            # btiles per collapse chunk
NCH = NT // NB                    # 4 chunks
NODES = 1024                      # 1023 real + 1 pad
F32 = mybir.dt.float32
F32R = mybir.dt.float32r
BF16 = mybir.dt.bfloat16
FP16 = mybir.dt.float16
I16 = mybir.dt.int16


def _build(nc, use_sign_path: bool):
    """Emit the per-core program. use_sign_path=True assumes b_pred == 0."""
    # hi/lo e8m11 split operands (fp32r is e8m11 on HW; hi+lo == fp32 exactly)
    xTh = nc.dram_tensor("xTh", [IN_DIM, B_LOC], F32R, kind="ExternalInput")
    xTl = nc.dram_tensor("xTl", [IN_DIM, B_LOC], F32R, kind="ExternalInput")
    Wph = nc.dram_tensor("Wph", [IN_DIM, NODES], F32R, kind="ExternalInput")
    Wpl = nc.dram_tensor("Wpl", [IN_DIM, NODES], F32R, kind="ExternalInput")
    Tout = nc.dram_tensor("Tout", [128, NODES], F32, kind="ExternalInput")
    Tstd = nc.dram_tensor("Tstd", [128, NODES], F32, kind="ExternalInput")
    TH = nc.dram_tensor("TH", [128, NODES], F32, kind="ExternalInput")
    Ident = nc.dram_tensor("Ident", [128, 128], F32, kind="ExternalInput")
    out_o = nc.dram_tensor("out_o", [B_LOC, OUT_DIM], F32, kind="ExternalOutput")
    out_s = nc.dram_tensor("out_s", [B_LOC, OUT_DIM], F32, kind="ExternalOutput")

    with tile.TileContext(nc) as tc, ExitStack() as ctx:
        consts = ctx.enter_context(tc.tile_pool(name="consts", bufs=1))
        xpool = ctx.enter_context(tc.tile_pool(name="xpool", bufs=4))
        spool = ctx.enter_context(tc.tile_pool(name="spool", bufs=3))
        rpool = ctx.enter_context(tc.tile_pool(name="rpool", bufs=3))
        dpool = ctx.enter_context(tc.tile_pool(name="dpool", bufs=3))
        zpool = ctx.enter_context(
            tc.tile_pool(name="zpool", bufs=3, space=bass.MemorySpace.PSUM)
        )
        tpool = ctx.enter_context(
            tc.tile_pool(name="tpool", bufs=2, space=bass.MemorySpace.PSUM)
        )

        wh = [
            consts.tile([128, NODES], F32R, tag=f"wh{k}", name=f"wh{k}")
            for k in range(2)
        ]
        wl = [
            consts.tile([128, NODES], F32R, tag=f"wl{k}", name=f"wl{k}")
            for k in range(2)
        ]
        for k in range(2):
            ks = slice(128 * k, 128 * (k + 1))
            nc.sync.dma_start(out=wh[k], in_=Wph[ks, :])
            nc.sync.dma_start(out=wl[k], in_=Wpl[ks, :])
        t_out = consts.tile([128, NODES], F32)
        t_std = consts.tile([128, NODES], F32)
        ident = consts.tile([128, 128], F32)
        th = None
        if not use_sign_path:
            th = consts.tile([128, NODES], F32)
            nc.sync.dma_start(out=th, in_=TH[:, :])

        def load_late_consts():
            # tables/identity are first consumed by the descent/output stage;
            # loading them after the first chunk's x keeps the PE start early.
            nc.sync.dma_start(out=t_out, in_=Tout[:, :])
            nc.sync.dma_start(out=t_std, in_=Tstd[:, :])
            nc.sync.dma_start(out=ident, in_=Ident[:, :])

        leaf_all = consts.tile([128, NT], FP16)
        leaf_i16 = consts.tile([128, NT], I16)
        r_out = consts.tile([128, NODES], F32)
        r_std = consts.tile([128, NODES], F32)

        o_view = out_o.rearrange("(t p f) c -> t p (f c)", t=8, p=128, f=8)
        s_view = out_s.rearrange("(t p f) c -> t p (f c)", t=8, p=128, f=8)
        LAG = 3

        def emit_out_chain(cc):
            # transpose chunk cc's gathered [128, 128] table blocks and DMA
            # them out; emitted LAG chunks late so the in-order PE queue
            # never stalls on the descent chain.
            rs_ = slice(128 * cc, 128 * (cc + 1))
            for rbuf, dview in ((r_out, o_view), (r_std, s_view)):
                pt = tpool.tile([128, 128], F32, tag="t", name="pt")
                nc.tensor.transpose(pt, rbuf[:, rs_], ident)
                rt = xpool.tile([128, 128], F32, tag="rt", name="rt", bufs=2)
                nc.scalar.copy(out=rt, in_=pt)
                nc.sync.dma_start(out=dview[cc], in_=rt)

        for c in range(NCH):
            # btile-MAJOR u-bit store: eviction writes [128, 1024] contiguous
            # (strided 2-byte writes cost ~4x on DVE; reads don't).
            s_chunk = spool.tile([128, NB, NODES], FP16, tag="s")
            for k in range(NB):
                t = c * NB + k
                bs = slice(128 * t, 128 * (t + 1))
                if k == 0:
                    # stage x for this chunk: [128, 128*NB] per ktile/half
                    hs = slice(128 * NB * c, 128 * NB * (c + 1))
                    xh = [
                        xpool.tile(
                            [128, 128 * NB], F32R,
                            tag=f"xh{kk}", name=f"xh{kk}", bufs=2,
                        )
                        for kk in range(2)
                    ]
                    xl = [
                        xpool.tile(
                            [128, 128 * NB], F32R,
                            tag=f"xl{kk}", name=f"xl{kk}", bufs=2,
                        )
                        for kk in range(2)
                    ]
                    for kk in range(2):
                        ks = slice(128 * kk, 128 * (kk + 1))
                        nc.sync.dma_start(out=xh[kk], in_=xTh[ks, hs])
                        nc.sync.dma_start(out=xl[kk], in_=xTl[ks, hs])
                    if c == 0:
                        load_late_consts()
                kb = slice(128 * k, 128 * (k + 1))
                z = zpool.tile([128, NODES], F32, tag="z")
                # z = xh@wh + xh@wl + xl@wh  (xl@wl term negligible)
                pair = 0
                for kk in range(2):
                    for lhs, rhs in (
                        (xh[kk], wh[kk]),
                        (xh[kk], wl[kk]),
                        (xl[kk], wh[kk]),
                    ):
                        for nh in range(2):
                            ns = slice(512 * nh, 512 * (nh + 1))
                            nc.tensor.matmul(
                                z[:, ns],
                                lhs[:, kb],
                                rhs[:, ns],
                                start=(pair == 0),
                                stop=(pair == 5),
                            )
                        pair += 1
                # u = (z < -b_pred); contiguous [128, 1024] write.
                # Explicit DVE/ACT split: ACT eviction uses the saturated
                # sigmoid trick u = Sigmoid(-1e30 * z) which is exactly
                # {0, 1} fp for any |z| > 1e-28.
                if use_sign_path:
                    if k % 8 < 4:
                        nc.scalar.activation(
                            out=s_chunk[:, k, :],
                            in_=z[:, :],
                            func=mybir.ActivationFunctionType.Sigmoid,
                            scale=-1e30,
                        )
                    else:
                        nc.vector.tensor_scalar(
                            out=s_chunk[:, k, :],
                            in0=z[:, :],
                            scalar1=0.0,
                            scalar2=None,
                            op0=mybir.AluOpType.is_lt,
                        )
                else:
                    nc.vector.tensor_tensor(
                        out=s_chunk[:, k, :],
                        in0=z[:, :],
                        in1=th[:, :],
                        op=mybir.AluOpType.is_lt,
                    )

            # ---- bottom-up collapse (fp16; all WRITES contiguous) ----
            # r_9 = u at level-9 nodes (columns 511..1022)
            r_prev = s_chunk[:, :, 511:1023]
            for i in range(8, -1, -1):
                n = 1 << i
                kconst = float(1 << (9 - i))
                u_i = s_chunk[:, :, n - 1 : 2 * n - 1]
                rp = r_prev.rearrange("p b (n two) -> p b n two", two=2)
                r_e = rp[:, :, :, 0]
                r_o = rp[:, :, :, 1]
                d_t = dpool.tile([128, NB, n], FP16, tag="d")
                # D = (r_o + K) - r_e
                nc.vector.scalar_tensor_tensor(
                    out=d_t,
                    in0=r_o,
                    scalar=kconst,
                    in1=r_e,
                    op0=mybir.AluOpType.add,
                    op1=mybir.AluOpType.subtract,
                )
                # D *= u
                nc.vector.tensor_tensor(
                    out=d_t, in0=u_i, in1=d_t, op=mybir.AluOpType.mult
                )
                # r = r_e + D
                if i > 0:
                    r_t = rpool.tile([128, NB, n], FP16, tag="r")
                    nc.vector.tensor_tensor(
                        out=r_t, in0=r_e, in1=d_t, op=mybir.AluOpType.add
                    )
                    r_prev = r_t
                else:
                    nc.vector.tensor_tensor(
                        out=leaf_all[:, c * NB : (c + 1) * NB],
                        in0=r_e[:, :, 0],
                        in1=d_t[:, :, 0],
                        op=mybir.AluOpType.add,
                    )

            # ---- per-chunk output stage ----
            # leaf -> int16. Sample rows are host-permuted within each
            # 128-block by pi(p) = 8*(p%16) + p//16, so leaf_i16 is already
            # in ap_gather's wrapped index layout and outputs land in
            # natural row order.
            cslice = slice(NB * c, NB * (c + 1))
            nc.vector.tensor_copy(
                out=leaf_i16[:, cslice], in_=leaf_all[:, cslice]
            )
            # table gathers: R[16g+cls, j] = T[cls, leaf(sample 8j+g)]
            rs = slice(128 * c, 128 * (c + 1))
            for tbl, rbuf in ((t_out, r_out), (t_std, r_std)):
                nc.gpsimd.ap_gather(
                    out_ap=rbuf[:, rs],
                    in_ap=tbl,
                    idxs_ap=leaf_i16[:, cslice],
                    channels=128,
                    num_elems=NODES,
                    d=1,
                    num_idxs=128,
                )
            if c >= LAG:
                emit_out_chain(c - LAG)

        for c in range(NCH - LAG, NCH):
            emit_out_chain(c)

    nc.compile()
    return nc


_CACHE = {}


def _get_nc(use_sign_path: bool):
    key = use_sign_path
    if key not in _CACHE:
        nc = bacc.Bacc("TRN2", target_bir_lowering=False, debug=False)
        _CACHE[key] = _build(nc, use_sign_path)
    return _CACHE[key]


# Within each 128-row block, device partition p holds sample row PERM[p].
# PERM aligns the collapse output with ap_gather's wrapped index layout and
# makes the final outputs land in natural row order (see kernel() docstring).
PERM = np.array([8 * (p % 16) + p // 16 for p in range(128)], dtype=np.int64)


def _e8m11(x):
    """Round fp32 to the HW fp32r format (8-bit exp, 11-bit mantissa, RNE)."""
    u = np.ascontiguousarray(x, np.float32).view(np.uint32)
    low = u & np.uint32(0xFFF)
    base = u & np.uint32(0xFFFFF000)
    add = (low > 0x800) | ((low == 0x800) & ((u >> 12) & 1).astype(bool))
    return (base + np.where(add, np.uint32(0x1000), np.uint32(0))).view(np.float32)


def _split_hi_lo(a):
    hi = _e8m11(a)
    lo = (a - hi).astype(np.float32)  # exactly e8m11-representable
    return hi, lo


def _shard_xT(x_shard):
    """[8192, 256] sample rows -> permuted, transposed [256, 8192] device input."""
    xp = x_shard.reshape(NT, 128, IN_DIM)[:, PERM, :].reshape(B_LOC, IN_DIM)
    return np.ascontiguousarray(xp.T)


def _prepare(x, W_pred, b_pred, W_or, action_stds):
    x = np.ascontiguousarray(x, dtype=np.float32)
    W_pred = np.asarray(W_pred, dtype=np.float32)
    b_pred = np.asarray(b_pred, dtype=np.float32)
    W_or = np.asarray(W_or, dtype=np.float32)
    action_stds = np.asarray(action_stds, dtype=np.float32)

    n_int = 2**HEIGHT - 1
    Wp = np.zeros((IN_DIM, NODES), np.float32)
    Wp[:, :n_int] = W_pred.T
    Wph, Wpl = _split_hi_lo(Wp)
    # softmax over classes per leaf column
    m = W_or.max(axis=0, keepdims=True)
    e = np.exp(W_or - m)
    t_out16 = (e / e.sum(axis=0, keepdims=True)).astype(np.float32)  # [16, 1024]
    t_std16 = np.clip(action_stds, -20.0, 2.0).astype(np.float32)
    t_out = np.tile(t_out16, (8, 1))  # [128, 1024]
    t_std = np.tile(t_std16, (8, 1))
    th16 = np.zeros((NODES,), np.float32)
    th16[:n_int] = -b_pred
    th = np.tile(th16[None, :], (128, 1))
    return x, Wph, Wpl, t_out, t_std, th, bool(np.any(b_pred != 0.0))


def kernel(x, W_pred, b_pred, W_or, action_stds, _want_trace=False):
    x, Wph, Wpl, t_out, t_std, th, b_nonzero = _prepare(
        x, W_pred, b_pred, W_or, action_stds
    )
    nc = _get_nc(use_sign_path=not b_nonzero)

    in_maps = []
    for c in range(N_CORES):
        shard = x[c * B_LOC : (c + 1) * B_LOC]
        xt = _shard_xT(shard)
        xth, xtl = _split_hi_lo(xt)
        in_maps.append(
            {
                "xTh": xth,
                "xTl": xtl,
                "Wph": Wph,
                "Wpl": Wpl,
                "Tout": t_out,
                "Tstd": t_std,
                "TH": th,
                "Ident": np.eye(128, dtype=np.float32),
            }
        )

    res = run_bass_kernel_spmd(
        nc, in_maps, core_ids=list(range(N_CORES)), trace=_want_trace
    )
    out = np.concatenate([res.results[c]["out_o"] for c in range(N_CORES)], axis=0)
    std = np.concatenate([res.results[c]["out_s"] for c in range(N_CORES)], axis=0)
    if _want_trace:
        kernel.last_results = res
    return out, std


# revision 35
# speedup vs baseline: 1.0228x; 1.0228x over previous
"""Trainium2 Bass kernel for nn_DGT_6485400616966 (soft decision tree forward).

Math (forward pass only):
  pred_z = x @ W_pred.T + b_pred                      [B, 1023]
  The straight-through/one-hot structure collapses: the output depends only on
  the argmax leaf of the tree AND layer, which equals a 10-level tree descent
  following sign(pred_z) at visited nodes (left if z >= 0).
  out = softmax(W_or[:, leaf]) ; std = clip(action_stds[:, leaf], -20, 2)

Device algorithm per core (8192 samples, data-parallel over 8 cores):
  1. PE: z = x @ W_pred.T in three fp32r passes (xh@wh + xh@wl + xl@wh) where
     hi/lo are an exact e8m11 split of the fp32 operands (fp32r on HW is
     e8m11; one pass alone flips ~38 argmax rows, three passes flip none).
     x tiles are the stationary operand; W^T columns (nodes, padded to 1024)
     are the moving operand, N=512 per matmul for full fp32r rate.
  2. Eviction PSUM->SBUF per btile: u = (z < 0) as fp16, contiguous writes
     (strided 2-byte DVE writes cost ~4x). Split DVE tensor_scalar is_lt /
     ACT saturated-sigmoid (Sigmoid(-1e30*z) is exactly {0,1}).
  3. DVE: bottom-up tree collapse r_i = r_e + u_i*(K + r_o - r_e) in fp16 on
     [128, NB, 2^i] chunk tensors (btile-major; all writes contiguous).
  4. GPSIMD ap_gather per chunk: table lookup T[class, leaf] with the 16
     classes replicated on partitions; each 16-partition group shares its
     sample's leaf index (host pre-permutes rows by pi(p)=8*(p%16)+p//16 so
     indices are already wrapped and outputs land in natural order).
  5. PE transpose of the gathered [128, 128] blocks (emitted LAG chunks late
     so the in-order PE queue never stalls) + contiguous DMA out.
"""

import sys

for _p in ("/opt/trn_rl_repo",):
    if _p not in sys.path:
        sys.path.insert(0, _p)

from contextlib import ExitStack

import numpy as np

import concourse.bacc as bacc
import concourse.bass as bass
import concourse.tile as tile
from concourse import mybir
from concourse.bass_utils import run_bass_kernel_spmd

HEIGHT = 10
IN_DIM = 256
OUT_DIM = 16
BATCH = 65536
N_CORES = 8
B_LOC = BATCH // N_CORES          # 8192 samples per core
NT = B_LOC // 128                 # 64 batch tiles of 128 samples
NB = 8                            # btiles per collapse chunk
NCH = NT // NB                    # 4 chunks
NODES = 1024                      # 1023 real + 1 pad
F32 = mybir.dt.float32
F32R = mybir.dt.float32r
BF16 = mybir.dt.bfloat16
FP16 = mybir.dt.float16
I16 = mybir.dt.int16


def _build(nc, use_sign_path: bool):
    """Emit the per-core program. use_sign_path=True assumes b_pred == 0."""
    # hi/lo e8m11 split operands (fp32r is e8m11 on HW; hi+lo == fp32 exactly)
    xTh = nc.dram_tensor("xTh", [IN_DIM, B_LOC], F32R, kind="ExternalInput")
    xTl = nc.dram_tensor("xTl", [IN_DIM, B_LOC], F32R, kind="ExternalInput")
    Wph = nc.dram_tensor("Wph", [IN_DIM, NODES], F32R, kind="ExternalInput")
    Wpl = nc.dram_tensor("Wpl", [IN_DIM, NODES], F32R, kind="ExternalInput")
    Tout = nc.dram_tensor("Tout", [128, NODES], F32, kind="ExternalInput")
    Tstd = nc.dram_tensor("Tstd", [128, NODES], F32, kind="ExternalInput")
    TH = nc.dram_tensor("TH", [128, NODES], F32, kind="ExternalInput")
    Ident = nc.dram_tensor("Ident", [128, 128], F32, kind="ExternalInput")
    out_o = nc.dram_tensor("out_o", [B_LOC, OUT_DIM], F32, kind="ExternalOutput")
    out_s = nc.dram_tensor("out_s", [B_LOC, OUT_DIM], F32, kind="ExternalOutput")

    with tile.TileContext(nc) as tc, ExitStack() as ctx:
        consts = ctx.enter_context(tc.tile_pool(name="consts", bufs=1))
        xpool = ctx.enter_context(tc.tile_pool(name="xpool", bufs=4))
        spool = ctx.enter_context(tc.tile_pool(name="spool", bufs=3))
        rpool = ctx.enter_context(tc.tile_pool(name="rpool", bufs=3))
        dpool = ctx.enter_context(tc.tile_pool(name="dpool", bufs=3))
        zpool = ctx.enter_context(
            tc.tile_pool(name="zpool", bufs=3, space=bass.MemorySpace.PSUM)
        )
        tpool = ctx.enter_context(
            tc.tile_pool(name="tpool", bufs=2, space=bass.MemorySpace.PSUM)
        )

        wh = [
            consts.tile([128, NODES], F32R, tag=f"wh{k}", name=f"wh{k}")
            for k in range(2)
        ]
        wl = [
            consts.tile([128, NODES], F32R, tag=f"wl{k}", name=f"wl{k}")
            for k in range(2)
        ]
        for k in range(2):
            ks = slice(128 * k, 128 * (k + 1))
            nc.sync.dma_start(out=wh[k], in_=Wph[ks, :])
            nc.sync.dma_start(out=wl[k], in_=Wpl[ks, :])
        t_out = consts.tile([128, NODES], F32)
        t_std = consts.tile([128, NODES], F32)
        ident = consts.tile([128, 128], F32)
        th = None
        if not use_sign_path:
            th = consts.tile([128, NODES], F32)
            nc.sync.dma_start(out=th, in_=TH[:, :])

        def load_late_consts():
            # tables/identity are first consumed by the descent/output stage;
            # loading them after the first chunk's x keeps the PE start early.
            nc.sync.dma_start(out=t_out, in_=Tout[:, :])
            nc.sync.dma_start(out=t_std, in_=Tstd[:, :])
            nc.sync.dma_start(out=ident, in_=Ident[:, :])

        leaf_all = consts.tile([128, NT], FP16)
        leaf_i16 = consts.tile([128, NT], I16)
        r_out = consts.tile([128, NODES], F32)
        r_std = consts.tile([128, NODES], F32)

        o_view = out_o.rearrange("(t p f) c -> t p (f c)", t=8, p=128, f=8)
        s_view = out_s.rearrange("(t p f) c -> t p (f c)", t=8, p=128, f=8)
        LAG = 3

        def emit_out_chain(cc):
            # transpose chunk cc's gathered [128, 128] table blocks and DMA
            # them out; emitted LAG chunks late so the in-order PE queue
            # never stalls on the descent chain.
            rs_ = slice(128 * cc, 128 * (cc + 1))
            for rbuf, dview in ((r_out, o_view), (r_std, s_view)):
                pt = tpool.tile([128, 128], F32, tag="t", name="pt")
                nc.tensor.transpose(pt, rbuf[:, rs_], ident)
                rt = xpool.tile([128, 128], F32, tag="rt", name="rt", bufs=2)
                nc.scalar.copy(out=rt, in_=pt)
                nc.sync.dma_start(out=dview[cc], in_=rt)

        for c in range(NCH):
            # btile-MAJOR u-bit store: eviction writes [128, 1024] contiguous
            # (strided 2-byte writes cost ~4x on DVE; reads don't).
            s_chunk = spool.tile([128, NB, NODES], FP16, tag="s")
            for k in range(NB):
                t = c * NB + k
                bs = slice(128 * t, 128 * (t + 1))
                if k == 0:
                    # stage x for this chunk: [128, 128*NB] per ktile/half
                    hs = slice(128 * NB * c, 128 * NB * (c + 1))
                    xh = [
                        xpool.tile(
                            [128, 128 * NB], F32R,
                            tag=f"xh{kk}", name=f"xh{kk}", bufs=2,
                        )
                        for kk in range(2)
                    ]
                    xl = [
                        xpool.tile(
                            [128, 128 * NB], F32R,
                            tag=f"xl{kk}", name=f"xl{kk}", bufs=2,
                        )
                        for kk in range(2)
                    ]
                    for kk in range(2):
                        ks = slice(128 * kk, 128 * (kk + 1))
                        nc.sync.dma_start(out=xh[kk], in_=xTh[ks, hs])
                        nc.sync.dma_start(out=xl[kk], in_=xTl[ks, hs])
                    if c == 0:
                        load_late_consts()
                kb = slice(128 * k, 128 * (k + 1))
                z = zpool.tile([128, NODES], F32, tag="z")
                # z = xh@wh + xh@wl + xl@wh  (xl@wl term negligible)
                pair = 0
                for kk in range(2):
                    for lhs, rhs in (
                        (xh[kk], wh[kk]),
                        (xh[kk], wl[kk]),
                        (xl[kk], wh[kk]),
                    ):
                        for nh in range(2):
                            ns = slice(512 * nh, 512 * (nh + 1))
                            nc.tensor.matmul(
                                z[:, ns],
                                lhs[:, kb],
                                rhs[:, ns],
                                start=(pair == 0),
                                stop=(pair == 5),
                            )
                        pair += 1
                # u = (z < -b_pred); contiguous [128, 1024] write.
                # Explicit DVE/ACT split: ACT eviction uses the saturated
                # sigmoid trick u = Sigmoid(-1e30 * z) which is exactly
                # {0, 1} fp for any |z| > 1e-28.
                if use_sign_path:
                    if k % 8 < 4:
                        nc.scalar.activation(
                            out=s_chunk[:, k, :],
                            in_=z[:, :],
                            func=mybir.ActivationFunctionType.Sigmoid,
                            scale=-1e30,
                        )
                    else:
                        nc.vector.tensor_scalar(
                            out=s_chunk[:, k, :],
                            in0=z[:, :],
                            scalar1=0.0,
                            scalar2=None,
                            op0=mybir.AluOpType.is_lt,
                        )
                else:
                    nc.vector.tensor_tensor(
                        out=s_chunk[:, k, :],
                        in0=z[:, :],
                        in1=th[:, :],
                        op=mybir.AluOpType.is_lt,
                    )

            # ---- bottom-up collapse (fp16; all WRITES contiguous) ----
            # r_9 = u at level-9 nodes (columns 511..1022)
            r_prev = s_chunk[:, :, 511:1023]
            for i in range(8, -1, -1):
                n = 1 << i
                kconst = float(1 << (9 - i))
                u_i = s_chunk[:, :, n - 1 : 2 * n - 1]
                rp = r_prev.rearrange("p b (n two) -> p b n two", two=2)
                r_e = rp[:, :, :, 0]
                r_o = rp[:, :, :, 1]
                d_t = dpool.tile([128, NB, n], FP16, tag="d")
                # D = (r_o + K) - r_e
                nc.vector.scalar_tensor_tensor(
                    out=d_t,
                    in0=r_o,
                    scalar=kconst,
                    in1=r_e,
                    op0=mybir.AluOpType.add,
                    op1=mybir.AluOpType.subtract,
                )
                # D *= u
                nc.vector.tensor_tensor(
                    out=d_t, in0=u_i, in1=d_t, op=mybir.AluOpType.mult
                )
                # r = r_e + D
                if i > 0:
                    r_t = rpool.tile([128, NB, n], FP16, tag="r")
                    nc.vector.tensor_tensor(
                        out=r_t, in0=r_e, in1=d_t, op=mybir.AluOpType.add
                    )
                    r_prev = r_t
                else:
                    nc.vector.tensor_tensor(
                        out=leaf_all[:, c * NB : (c + 1) * NB],
                        in0=r_e[:, :, 0],
                        in1=d_t[:, :, 0],
                        op=mybir.AluOpType.add,
                    )

            # ---- per-chunk output stage ----
            # leaf -> int16. Sample rows are host-permuted within each
            # 128-block by pi(p) = 8*(p%16) + p//16, so leaf_i16 is already
            # in ap_gather's wrapped index layout and outputs land in
            # natural row order.
            cslice = slice(NB * c, NB * (c + 1))
            nc.vector.tensor_copy(
                out=leaf_i16[:, cslice], in_=leaf_all[:, cslice]
            )
            # table gathers: R[16g+cls, j] = T[cls, leaf(sample 8j+g)]
            rs = slice(128 * c, 128 * (c + 1))
            for tbl, rbuf in ((t_out, r_out), (t_std, r_std)):
                nc.gpsimd.ap_gather(
                    out_ap=rbuf[:, rs],
                    in_ap=tbl,
                    idxs_ap=leaf_i16[:, cslice],
                    channels=128,
                    num_elems=NODES,
                    d=1,
                    num_idxs=128,
                )
            if c >= LAG:
                emit_out_chain(c - LAG)

        for c in range(NCH - LAG, NCH):
            emit_out_chain(c)

    nc.compile()
    return nc


_CACHE = {}


def _get_nc(use_sign_path: bool):
    key = use_sign_path
    if key not in _CACHE:
        nc = bacc.Bacc("TRN2", target_bir_lowering=False, debug=False)
        _CACHE[key] = _build(nc, use_sign_path)
    return _CACHE[key]


# Within each 128-row block, device partition p holds sample row PERM[p].
# PERM aligns the collapse output with ap_gather's wrapped index layout and
# makes the final outputs land in natural row order (see kernel() docstring).
PERM = np.array([8 * (p % 16) + p // 16 for p in range(128)], dtype=np.int64)


def _e8m11(x):
    """Round fp32 to the HW fp32r format (8-bit exp, 11-bit mantissa, RNE)."""
    u = np.ascontiguousarray(x, np.float32).view(np.uint32)
    low = u & np.uint32(0xFFF)
    base = u & np.uint32(0xFFFFF000)
    add = (low > 0x800) | ((low == 0x800) & ((u >> 12) & 1).astype(bool))
    return (base + np.where(add, np.uint32(0x1000), np.uint32(0))).view(np.float32)


def _split_hi_lo(a):
    hi = _e8m11(a)
    lo = (a - hi).astype(np.float32)  # exactly e8m11-representable
    return hi, lo


def _shard_xT(x_shard):
    """[8192, 256] sample rows -> permuted, transposed [256, 8192] device input."""
    xp = x_shard.reshape(NT, 128, IN_DIM)[:, PERM, :].reshape(B_LOC, IN_DIM)
    return np.ascontiguousarray(xp.T)


def _prepare(x, W_pred, b_pred, W_or, action_stds):
    x = np.ascontiguousarray(x, dtype=np.float32)
    W_pred = np.asarray(W_pred, dtype=np.float32)
    b_pred = np.asarray(b_pred, dtype=np.float32)
    W_or = np.asarray(W_or, dtype=np.float32)
    action_stds = np.asarray(action_stds, dtype=np.float32)

    n_int = 2**HEIGHT - 1
    Wp = np.zeros((IN_DIM, NODES), np.float32)
    Wp[:, :n_int] = W_pred.T
    Wph, Wpl = _split_hi_lo(Wp)
    # softmax over classes per leaf column
    m = W_or.max(axis=0, keepdims=True)
    e = np.exp(W_or - m)
    t_out16 = (e / e.sum(axis=0, keepdims=True)).astype(np.float32)  # [16, 1024]
    t_std16 = np.clip(action_stds, -20.0, 2.0).astype(np.float32)
    t_out = np.tile(t_out16, (8, 1))  # [128, 1024]
    t_std = np.tile(t_std16, (8, 1))
    th16 = np.zeros((NODES,), np.float32)
    th16[:n_int] = -b_pred
    th = np.tile(th16[None, :], (128, 1))
    return x, Wph, Wpl, t_out, t_std, th, bool(np.any(b_pred != 0.0))


def kernel(x, W_pred, b_pred, W_or, action_stds, _want_trace=False):
    x, Wph, Wpl, t_out, t_std, th, b_nonzero = _prepare(
        x, W_pred, b_pred, W_or, action_stds
    )
    nc = _get_nc(use_sign_path=not b_nonzero)

    in_maps = []
    for c in range(N_CORES):
        shard = x[c * B_LOC : (c + 1) * B_LOC]
        xt = _shard_xT(shard)
        xth, xtl = _split_hi_lo(xt)
        in_maps.append(
            {
                "xTh": xth,
                "xTl": xtl,
                "Wph": Wph,
                "Wpl": Wpl,
                "Tout": t_out,
                "Tstd": t_std,
                "TH": th,
                "Ident": np.eye(128, dtype=np.float32),
            }
        )

    res = run_bass_kernel_spmd(
        nc, in_maps, core_ids=list(range(N_CORES)), trace=_want_trace
    )
    out = np.concatenate([res.results[c]["out_o"] for c in range(N_CORES)], axis=0)
    std = np.concatenate([res.results[c]["out_s"] for c in range(N_CORES)], axis=0)
    if _want_trace:
        kernel.last_results = res
    return out, std
